# revision 9
# baseline (speedup 1.0000x reference)
"""MoE FFN layer (8 experts, top-2, SwiGLU) on 8 trn2 NeuronCores.

Strategy: expert parallelism with host-side routing. The gate (0.2% of the
FLOPs) plus top-k/softmax/combine run on host in float64; each core runs the
dense SwiGLU FFN of one expert over only the tokens routed to it (gathered
and transposed on host, padded to a common capacity C). That is 4x fewer
FLOPs than the dense formulation of the reference. The combine (scatter-add
weighted by the top-2 softmax) and the aux loss are host-side as well.

Device kernel per core (expert e):
  inputs  xT [128, KH, C] (tokens^T, pre-blocked), w1/w3 [KF, 128, KH, 128],
          w2 [KH, 128, KF, 128]  (f32, pre-blocked per 128x128 PE tile)
  output  yT [H, C]
  phase A: h1T/h3T [F, C] = w1^T x / w3^T x   (PE, float32r, PSUM f32 accum)
           actT = silu(h1T) * h3T             (ACT + DVE)
  phase C: yT = w2^T actT                     (PE, float32r)
Matmuls run in float32r (TF32-like, ~1.6e-4 rel err per matmul) which
streams at 1 row/cycle for free-dim >= 256 vs fp32's 4 cycles/row.
Weight tiles are pre-blocked on host so each tile loads as ONE fully
contiguous DMA; DMA issue is spread over the SP/ACT/DVE HWDGE queues.
"""
import os
import sys

for _p in (
    "/root/.axon_site",
    "/root/.axon_site/_ro/trn_rl_repo",
    "/root/.axon_site/_ro/pypackages",
    "/opt/trn_rl_repo",
    "/opt/pypackages",
):
    if os.path.isdir(_p) and _p not in sys.path:
        sys.path.append(_p)

import numpy as np

import concourse.bass as bass
import concourse.mybir as mybir
import concourse.tile as tile
from concourse import bacc

H = 1024
F = 2816
E = 8
TOP_K = 2
TEMPERATURE = 1.0
LB_WEIGHT = 0.01
KH = H // 128   # 8  k-tiles over hidden
KF = F // 128   # 22 k-tiles over ffn dim

F32 = mybir.dt.float32
F32R = mybir.dt.float32r
SILU = mybir.ActivationFunctionType.Silu


def _chunks(c):
    """Split the token free-dim into PSUM-bank-aligned (<=512) chunks."""
    out, o = [], 0
    while c - o > 512:
        out.append((o, 512))
        o += 512
    out.append((o, c - o))
    return out


def build_program(C, loop=0, probe=""):
    """Build the per-core Bass program for capacity C. loop>0 wraps the body
    in a For_i repeat (for wall-clock delta timing only). probe: "nodma"
    drops the weight/x loads, "nomm" drops the matmuls (timing probes only)."""
    nc = bacc.Bacc("TRN2", target_bir_lowering=False, debug=False)

    xT_d = nc.dram_tensor("xT", [128, KH, C], F32, kind="ExternalInput")
    w1_d = nc.dram_tensor("w1", [KF, 128, KH, 128], F32, kind="ExternalInput")
    w3_d = nc.dram_tensor("w3", [KF, 128, KH, 128], F32, kind="ExternalInput")
    w2_d = nc.dram_tensor("w2", [KH, 128, KF, 128], F32, kind="ExternalInput")
    yT_d = nc.dram_tensor("yT", [H, C], F32, kind="ExternalOutput")

    xb = xT_d.ap().bitcast(F32R)
    w1b = w1_d.ap().bitcast(F32R)
    w3b = w3_d.ap().bitcast(F32R)
    w2b = w2_d.ap().bitcast(F32R)
    yr = yT_d.ap().rearrange("(k p) c -> k p c", p=128)

    cks = _chunks(C)

    with tile.TileContext(nc) as tc:
        with (
            tc.tile_pool(name="xpool", bufs=1) as xpool,
            tc.tile_pool(name="wpool", bufs=3) as wpool,
            tc.tile_pool(name="apool", bufs=1) as apool,
            tc.tile_pool(name="tpool", bufs=3) as tpool,
            tc.tile_pool(name="opool", bufs=3) as opool,
            tc.tile_pool(name="psA", bufs=1, space="PSUM") as psA,
            tc.tile_pool(name="psC", bufs=2, space="PSUM") as psC,
        ):

            do_dma = probe != "nodma"
            do_mm = probe != "nomm"

            def body(_i=None):
                xT_sb = xpool.tile([128, KH, C], F32R, name="xT_sb")
                if do_dma:
                    nc.sync.dma_start(out=xT_sb[:], in_=xb[:])

                act_tiles = []
                for fi in range(KF):
                    w1_t = wpool.tile([128, KH, 128], F32R, tag="w1", name="w1_t")
                    w3_t = wpool.tile([128, KH, 128], F32R, tag="w3", name="w3_t")
                    if do_dma:
                        nc.sync.dma_start(out=w1_t[:], in_=w1b[fi])
                        nc.scalar.dma_start(out=w3_t[:], in_=w3b[fi])
                    h1_ps = psA.tile([128, C], F32, tag="h1", name="h1_ps")
                    h3_ps = psA.tile([128, C], F32, tag="h3", name="h3_ps")
                    if do_mm:
                        for k in range(KH):
                            for c0, cn in cks:
                                nc.tensor.matmul(
                                    h1_ps[:, c0:c0 + cn], w1_t[:, k, :],
                                    xT_sb[:, k, c0:c0 + cn],
                                    start=(k == 0), stop=(k == KH - 1),
                                )
                        for k in range(KH):
                            for c0, cn in cks:
                                nc.tensor.matmul(
                                    h3_ps[:, c0:c0 + cn], w3_t[:, k, :],
                                    xT_sb[:, k, c0:c0 + cn],
                                    start=(k == 0), stop=(k == KH - 1),
                                )
                    else:
                        nc.gpsimd.memset(h1_ps[:], 0.0)
                        nc.gpsimd.memset(h3_ps[:], 0.0)
                    sil = tpool.tile([128, C], F32, tag="sil", name="sil")
                    nc.scalar.activation(sil[:], h1_ps[:], SILU)
                    a_t = apool.tile([128, C], F32R, tag=f"a{fi}", name=f"a{fi}")
                    nc.vector.tensor_mul(a_t[:], sil[:], h3_ps[:])
                    act_tiles.append(a_t)

                for hi in range(KH):
                    w2_t = wpool.tile([128, KF, 128], F32R, tag="w2", bufs=2,
                                      name="w2_t")
                    if do_dma:
                        nc.gpsimd.dma_start(out=w2_t[:], in_=w2b[hi])
                    y_ps = psC.tile([128, C], F32, tag="y", name="y_ps")
                    if do_mm:
                        for k in range(KF):
                            for c0, cn in cks:
                                nc.tensor.matmul(
                                    y_ps[:, c0:c0 + cn], w2_t[:, k, :],
                                    act_tiles[k][:, c0:c0 + cn],
                                    start=(k == 0), stop=(k == KF - 1),
                                )
                    else:
                        nc.gpsimd.memset(y_ps[:], 0.0)
                    y_sb = opool.tile([128, C], F32, tag="y_sb", name="y_sb")
                    nc.vector.tensor_copy(y_sb[:], y_ps[:])
                    nc.sync.dma_start(out=yr[hi], in_=y_sb[:])

            if loop > 0:
                with tc.For_i(0, loop, 1):
                    body()
            else:
                body()

    nc.finalize()
    return nc


def _route(x, gate_w):
    """Host-side gating in float64. Returns top2 idx [T,2], weights [T,2],
    aux_loss (np.float32)."""
    T = x.shape[0]
    logits = x.astype(np.float64) @ gate_w.astype(np.float64).T   # [T, E]
    order = np.argsort(-logits, axis=-1, kind="stable")
    top2 = order[:, :TOP_K]                                       # [T, 2]
    vals = np.take_along_axis(logits / TEMPERATURE, top2, axis=-1)
    vals = vals - vals.max(axis=-1, keepdims=True)
    ex = np.exp(vals)
    wts = ex / ex.sum(axis=-1, keepdims=True)                     # [T, 2]

    # aux load-balancing loss
    gl = logits - logits.max(axis=-1, keepdims=True)
    gp = np.exp(gl)
    gp /= gp.sum(axis=-1, keepdims=True)                          # [T, E]
    usage = np.bincount(top2.ravel(), minlength=E).astype(np.float64)
    usage = usage / (T * TOP_K + 1e-9)
    importance = gp.mean(axis=0)
    aux = min(float((usage * importance).sum() * E) * LB_WEIGHT, 1.0)
    return top2, wts, np.float32(aux)


def _block_x(xe_pad):
    """[H, C] -> [128, KH, C] so the device can load it in one DMA."""
    return np.ascontiguousarray(
        xe_pad.reshape(KH, 128, -1).transpose(1, 0, 2))


def _block_w13(w):
    """[H, F] -> [KF, 128, KH, 128]: per f-tile contiguous lhsT blocks."""
    return np.ascontiguousarray(
        w.reshape(KH, 128, KF, 128).transpose(2, 1, 0, 3))


def _block_w2(w):
    """[F, H] -> [KH, 128, KF, 128]: per h-tile contiguous lhsT blocks."""
    return np.ascontiguousarray(
        w.reshape(KF, 128, KH, 128).transpose(2, 1, 0, 3))


def make_in_maps(xf, idx, C, w1, w3, w2):
    in_maps = []
    for e in range(E):
        xT = np.zeros((H, C), dtype=np.float32)
        xT[:, : len(idx[e])] = xf[idx[e]].T
        in_maps.append({
            "xT": _block_x(xT),
            "w1": _block_w13(np.asarray(w1[e], dtype=np.float32)),
            "w3": _block_w13(np.asarray(w3[e], dtype=np.float32)),
            "w2": _block_w2(np.asarray(w2[e], dtype=np.float32)),
        })
    return in_maps


_progs = {}


def _get_program(C, loop=0):
    key = (C, loop)
    if key not in _progs:
        _progs[key] = build_program(C, loop=loop)
    return _progs[key]


def _ffn_host(xe, w1e, w3e, w2e):
    """fp32 SwiGLU FFN on host for the few tokens beyond device capacity."""
    h1 = xe @ w1e
    h3 = xe @ w3e
    act = (h1 / (1.0 + np.exp(-h1))) * h3
    return act @ w2e


def kernel(x, gate_w, w1, w3, w2):
    B, S, _ = x.shape
    xf = np.ascontiguousarray(np.asarray(x, dtype=np.float32).reshape(-1, H))
    T = xf.shape[0]

    top2, wts, aux = _route(xf, np.asarray(gate_w, dtype=np.float32))

    full_idx = [np.where((top2 == e).any(axis=1))[0] for e in range(E)]
    n_max = max(len(i) for i in full_idx)
    # Device capacity: one full PSUM bank (512) keeps every matmul a single
    # max-size instruction; the few tokens beyond 512 per expert run on host.
    C = min(512, max(256, -(-n_max // 32) * 32))
    idx = [i[:C] for i in full_idx]
    over = [i[C:] for i in full_idx]

    in_maps = make_in_maps(xf, idx, C, w1, w3, w2)

    from concourse.bass_utils import run_bass_kernel_spmd

    results = run_bass_kernel_spmd(
        _get_program(C), in_maps, list(range(E))
    ).results

    out = np.zeros((T, H), dtype=np.float32)
    for e in range(E):
        ids = idx[e]
        if len(ids):
            ye = results[e]["yT"][:, : len(ids)].T            # [n_e, H]
            sel = top2[ids] == e                              # [n_e, 2]
            we = (wts[ids] * sel).sum(axis=1).astype(np.float32)
            out[ids] += we[:, None] * ye
        oids = over[e]
        if len(oids):
            yo = _ffn_host(xf[oids],
                           np.asarray(w1[e], dtype=np.float32),
                           np.asarray(w3[e], dtype=np.float32),
                           np.asarray(w2[e], dtype=np.float32))
            sel = top2[oids] == e
            we = (wts[oids] * sel).sum(axis=1).astype(np.float32)
            out[oids] += we[:, None] * yo

    return out.reshape(B, S, H), aux
